# revision 1
# baseline (speedup 1.0000x reference)
"""Trainium2 Bass kernel for the Chambolle-Pock-style primal/dual stencil loop.

Math (per image, H=W=1024, EPS=0.5, TAU=0.5, 10 iterations):
    u = sigmoid(o/EPS); q = 0
    repeat 10x:
        q  = relu(q - TAU*(vf1*Dy(u) + vf0*Dx(u)))   # forward diffs, zero pad
        Tq = BDy(vf1*q) + BDx(vf0*q)                  # backward diffs, zero pad
        u  = sigmoid((o - Tq)/EPS)
    return (o - Tq)/EPS

Rescaling trick: with qh = 2*sqrt(2)*q, g = vf/sqrt(2) (host-side) and
s = 2(o - Tq), and representing u through t = tanh(s/2) (u = 0.5 + 0.5t, the
0.5s cancel in every stencil difference; zero-padding of u becomes
(-1)-padding of t):
    qh = relu(qh - (g1*Dy(t) + g0*Dx(t)))            # t pads: -1
    s  = o2 - BDy(g1*qh) - BDx(g0*qh)                # o2 = 2*o, pads 0
    t  = tanh(s/2)
and the final output is s itself.  tanh is used instead of sigmoid because its
activation table is ~10x more accurate (4 vs 40 ULP) and the relu makes
isolated pixels chaotic under any per-step rounding noise; everything else is
kept in exact fp32 for the same reason (measured: rel-L2 vs the fp32 jax
reference is ~1e-5, max-abs ~0.028 — the fp32 reference's own fp64 envelope).

Sharding: pure data parallel, one image per NeuronCore (B=8 over 8 cores),
vf0/vf1 broadcast to all cores.

Layout: image row y = 8*p + i -> partition p (0..127), plane i (0..7) in the
free dimension.  A +1 row shift is then a free-dim plane offset for i<7; only
the plane-7 -> next-partition boundary needs a cross-partition move, done with
a tiny SBUF->SBUF DMA into a 9th plane.  Column shifts use guard columns.
The whole working set (6 image buffers, ~200KB/partition) stays SBUF resident,
so HBM traffic is one 12MB load + 4MB store per core.  All elementwise ops run
on VectorE split into plane-halves so relu/tanh (ScalarE) and the boundary
DMAs overlap the VectorE stream.
"""

import numpy as np

import concourse.bacc as bacc
import concourse.mybir as mybir
from concourse.tile import TileContext
from concourse import bass_utils

F32 = mybir.dt.float32
AF = mybir.ActivationFunctionType

B, H, W = 8, 1024, 1024
P = 128          # SBUF partitions
NP = H // P      # planes per partition = 8
WG = W + 1       # plane width incl. one guard column
MAXITER = 10

_CACHE = {}
LAST_RESULTS = None  # BassKernelResults of the most recent run (for test.py)


def _build(reps=1):
    """Build the Bass program.  reps>1 repeats the whole computation (state
    re-initialized each rep, same output) — used only for wall-clock timing
    of the HW kernel when no NTFF profiling is available."""
    nc = bacc.Bacc("TRN2", target_bir_lowering=False, debug=False)

    o2_d = nc.dram_tensor("o2", [H, W], F32, kind="ExternalInput").ap()
    g0_d = nc.dram_tensor("g0", [H, W], F32, kind="ExternalInput").ap()
    g1_d = nc.dram_tensor("g1", [H, W], F32, kind="ExternalInput").ap()
    out_d = nc.dram_tensor("out", [H, W], F32, kind="ExternalOutput").ap()

    # (H, W) -> (p, i, x) with y = 8*p + i
    o2_v = o2_d.rearrange("(p i) x -> p i x", i=NP)
    g0_v = g0_d.rearrange("(p i) x -> p i x", i=NP)
    g1_v = g1_d.rearrange("(p i) x -> p i x", i=NP)
    out_v = out_d.rearrange("(p i) x -> p i x", i=NP)

    v = nc.vector
    act = nc.scalar

    with TileContext(nc) as tc:
        with tc.tile_pool(name="main", bufs=1) as pool:
            o2t = pool.tile([P, NP, W], F32)
            g0t = pool.tile([P, NP, W], F32)
            g1t = pool.tile([P, NP, W], F32)
            qht = pool.tile([P, NP, W], F32)
            # su: planes 0..7 = t/s data (col W = -1 guard for x+1 reads),
            # plane 8 = boundary row t[8p+8, x] (partition 127 stays -1)
            sut = pool.tile([P, NP + 1, WG], F32)
            # tmp: planes 1..8 = a/b scratch at cols 1..W (col 0 = zero guard
            # for x-1 reads), plane 0 = boundary row a[8p-1, x]
            tmpt = pool.tile([P, NP + 1, WG], F32)

            halves = [(0, NP // 2), (NP // 2, NP)]

            def u_(lo, hi):
                return sut[:, lo:hi, 0:W]

            def unr(lo, hi):   # t[y+1, x] (plane 8 = boundary)
                return sut[:, lo + 1 : hi + 1, 0:W]

            def unc(lo, hi):   # t[y, x+1] (col W = -1 guard)
                return sut[:, lo:hi, 1 : W + 1]

            def t_(lo, hi):
                return tmpt[:, lo + 1 : hi + 1, 1 : W + 1]

            def tpr(lo, hi):   # a[y-1, x] (plane 0 = boundary)
                return tmpt[:, lo:hi, 1 : W + 1]

            def tpc(lo, hi):   # b[y, x-1] (col 0 = zero guard)
                return tmpt[:, lo + 1 : hi + 1, 0:W]

            def o2_(lo, hi):
                return o2t[:, lo:hi, :]

            def g0_(lo, hi):
                return g0t[:, lo:hi, :]

            def g1_(lo, hi):
                return g1t[:, lo:hi, :]

            def qh_(lo, hi):
                return qht[:, lo:hi, :]

            # --- setup ---
            # t-state guards are -1 (tanh representation of u=0 padding).
            v.memset(sut[:, :, :], -1.0)
            v.memset(tmpt[:, :, :], 0.0)  # zero guards + a-boundary row 0
            nc.sync.dma_start(out=o2t[:, :, :], in_=o2_v)
            nc.sync.dma_start(out=g0t[:, :, :], in_=g0_v)
            nc.sync.dma_start(out=g1t[:, :, :], in_=g1_v)

            def dma_ushift():
                # su[p, 8, x] = t[8p+8, x] = su[p+1, 0, x]; row 127 stays -1
                nc.sync.dma_start(
                    out=sut[0 : P - 1, NP, 0:W], in_=sut[1:P, 0, 0:W]
                )

            def dma_ashift():
                # tmp[p, 0, c] = a[8p-1] = tmp[p-1, 8, c]; row 0 stays 0
                nc.sync.dma_start(
                    out=tmpt[1:P, 0, 1 : W + 1], in_=tmpt[0 : P - 1, NP, 1 : W + 1]
                )

            for _rep in range(reps):
                if reps > 1:
                    v.memset(sut[:, :, :], -1.0)
                v.memset(qht[:, :, :], 0.0)
                for lo, hi in halves:
                    act.activation(u_(lo, hi), o2_(lo, hi), AF.Tanh, scale=0.5)
                dma_ushift()

                for it in range(MAXITER):
                    last = it == MAXITER - 1
                    # dual: qh = relu(qh - g1*Dy(t) - g0*Dx(t))
                    for lo, hi in halves:
                        v.tensor_sub(t_(lo, hi), unr(lo, hi), u_(lo, hi))
                        v.tensor_mul(t_(lo, hi), t_(lo, hi), g1_(lo, hi))
                        v.tensor_sub(qh_(lo, hi), qh_(lo, hi), t_(lo, hi))
                    for lo, hi in halves:
                        v.tensor_sub(t_(lo, hi), unc(lo, hi), u_(lo, hi))
                        v.tensor_mul(t_(lo, hi), t_(lo, hi), g0_(lo, hi))
                        v.tensor_sub(qh_(lo, hi), qh_(lo, hi), t_(lo, hi))
                        act.activation(qh_(lo, hi), qh_(lo, hi), AF.Relu)
                    # primal: s = o2 - (a-a_pr) - (b-b_pc), a = g1*qh, b = g0*qh
                    # upper a-half first so the boundary-row DMA fires early
                    v.tensor_mul(t_(*halves[1]), g1_(*halves[1]), qh_(*halves[1]))
                    dma_ashift()
                    v.tensor_mul(t_(*halves[0]), g1_(*halves[0]), qh_(*halves[0]))
                    for lo, hi in halves:
                        v.tensor_sub(u_(lo, hi), o2_(lo, hi), t_(lo, hi))
                        v.tensor_add(u_(lo, hi), u_(lo, hi), tpr(lo, hi))
                    for lo, hi in halves:
                        v.tensor_mul(t_(lo, hi), g0_(lo, hi), qh_(lo, hi))
                        v.tensor_sub(u_(lo, hi), u_(lo, hi), t_(lo, hi))
                        v.tensor_add(u_(lo, hi), u_(lo, hi), tpc(lo, hi))
                        if not last:
                            act.activation(
                                u_(lo, hi), u_(lo, hi), AF.Tanh, scale=0.5
                            )
                            if lo == 0:
                                dma_ushift()

            nc.sync.dma_start(out=out_v, in_=sut[:, 0:NP, 0:W])

    nc.compile()
    return nc


def kernel(o, vector_field, nabla_w, div_w):
    global LAST_RESULTS
    if "nc" not in _CACHE:
        _CACHE["nc"] = _build()
    nc = _CACHE["nc"]

    o2 = np.ascontiguousarray(2.0 * np.asarray(o, dtype=np.float32)[:, 0])
    vf = np.asarray(vector_field, dtype=np.float32)
    s = np.float32(1.0 / np.sqrt(2.0))
    g0 = np.ascontiguousarray(vf[:, :, 0] * s)
    g1 = np.ascontiguousarray(vf[:, :, 1] * s)

    in_maps = [{"o2": o2[b], "g0": g0, "g1": g1} for b in range(B)]
    res = bass_utils.run_bass_kernel_spmd(nc, in_maps, core_ids=list(range(B)))
    LAST_RESULTS = res
    return np.stack([r["out"] for r in res.results]).astype(np.float32)



# revision 4
# speedup vs baseline: 1.6974x; 1.6974x over previous
"""Trainium2 Bass kernel for the Chambolle-Pock-style primal/dual stencil loop.

Math (per image, H=W=1024, EPS=0.5, TAU=0.5, 10 iterations):
    u = sigmoid(o/EPS); q = 0
    repeat 10x:
        q  = relu(q - TAU*(vf1*Dy(u) + vf0*Dx(u)))   # forward diffs, zero pad
        Tq = BDy(vf1*q) + BDx(vf0*q)                  # backward diffs, zero pad
        u  = sigmoid((o - Tq)/EPS)
    return (o - Tq)/EPS

Rescaled (qh = 2*sqrt(2)*q, g = vf/sqrt(2), o2 = 2*o, t = tanh(s/2) with
s = 2(o - Tq); u-padding 0 becomes t-padding -1) and expanded so every
product has un-shifted operands where possible (gs = g0+g1, g1u = g1 shifted
down one row, host-precomputed):
    dual:   qh = relu(qh + gs*t - g1*t_dn - g0*t_rt)        # t_dn=t(y+1), t_rt=t(x+1)
    primal: s  = o2 - gs*qh + g1u*qh_u + (g0*qh)(x-1)       # qh_u=qh(y-1)
    t = tanh(s/2); output = s of the last iteration.

All state and arithmetic are fp16: the DVE runs tensor_tensor at 2 elem/cyc
for packed 2-byte operands, halving the per-pass cost vs fp32.  fp16 (not
bf16) because the relu makes isolated pixels chaotic under per-step rounding
noise: measured rel-L2 vs the fp32 reference is ~8e-3 for fp16 end-to-end but
~3e-2 for bf16 (over the 2e-2 gate).  Value ranges (|qh|<12, |s|<32) are far
from fp16 overflow.

Engine split: the two x-shifted passes (the C~ product reading t(x+1) and the
F accumulation s(x) += (g0*qh)(x-1)) have 2-byte-misaligned operands, which
breaks the DVE 2x perf mode's 4B-alignment requirement on real HW, so they
run on GPSIMD (alignment/dtype-indifferent, scalar_tensor_tensor at ~0.6
roofline efficiency beats plain TT's 0.42).  GPSIMD also takes the top
GP_SHARE planes of the A/D/E products for load balance: per iteration the
DVE does ~70 plane-passes at 0.55us and GPSIMD ~24 at 1.44us.  relu/tanh run
on the Scalar(Act) engine, overlapped with the DVE stream by processing the
image in plane-halves.

Sharding: pure data parallel, one image per NeuronCore (B=8 over 8 cores),
g-fields broadcast.  Layout: image row y = 8*p + i -> partition p (0..127),
plane i (0..7); +-1 row shifts are free-dim plane offsets with one guard
plane filled by a tiny SBUF->SBUF DMA per iteration; x+1 uses a guard column.
Whole working set (~180KB/partition) is SBUF resident: HBM traffic is one
10MB fp16 load + 2MB store per core.
"""

import numpy as np

import concourse.bacc as bacc
import concourse.mybir as mybir
from concourse.tile import TileContext
from concourse import bass_utils

F16 = mybir.dt.float16
AF = mybir.ActivationFunctionType
ALU = mybir.AluOpType

B, H, W = 8, 1024, 1024
P = 128          # SBUF partitions
NP = H // P      # planes per partition = 8
WG = W + 2       # t-plane width incl. guard column (even, keeps 4B align)
MAXITER = 10
GP_SHARE = 2     # top planes of A/D/E products computed on GPSIMD

_CACHE = {}
LAST_RESULTS = None  # BassKernelResults of the most recent run (for test.py)


def _build(reps=1):
    """Build the Bass program.  reps>1 repeats the whole computation (state
    re-initialized each rep, same output) — used only for wall-clock timing
    of the HW kernel when no NTFF profiling is available."""
    nc = bacc.Bacc("TRN2", target_bir_lowering=False, debug=False)

    o2_d = nc.dram_tensor("o2", [H, W], F16, kind="ExternalInput").ap()
    g0_d = nc.dram_tensor("g0", [H, W], F16, kind="ExternalInput").ap()
    g1_d = nc.dram_tensor("g1", [H, W], F16, kind="ExternalInput").ap()
    gs_d = nc.dram_tensor("gs", [H, W], F16, kind="ExternalInput").ap()
    g1u_d = nc.dram_tensor("g1u", [H, W], F16, kind="ExternalInput").ap()
    out_d = nc.dram_tensor("out", [H, W], F16, kind="ExternalOutput").ap()

    # (H, W) -> (p, i, x) with y = 8*p + i
    def vu(ap):
        return ap.rearrange("(p i) x -> p i x", i=NP)

    v = nc.vector
    gp = nc.gpsimd
    act = nc.scalar

    DV = NP - GP_SHARE  # planes 0..DV-1 of split products on DVE

    with TileContext(nc) as tc:
        with tc.tile_pool(name="main", bufs=1) as pool:
            o2t = pool.tile([P, NP, W], F16)
            g0t = pool.tile([P, NP, W], F16)
            g1t = pool.tile([P, NP, W], F16)
            gst = pool.tile([P, NP, W], F16)
            g1ut = pool.tile([P, NP, W], F16)
            st = pool.tile([P, NP, W], F16)
            # t-state: planes 0..7 = t at cols 0..W-1, col W = -1 guard for
            # x+1 reads, plane 8 = boundary row t[8p+8] (partition 127: -1)
            sut = pool.tile([P, NP + 1, WG], F16)
            # qh-state: planes 1..8 = qh rows 8p..8p+7, plane 0 = boundary
            # row qh[8p-1] (partition 0: zero pad)
            qht = pool.tile([P, NP + 1, W], F16)
            tA = pool.tile([P, NP, W], F16)
            tB = pool.tile([P, NP, W], F16)
            tC = pool.tile([P, NP, W], F16)

            halves = [(0, NP // 2), (NP // 2, NP)]

            def t_(lo, hi):     # t rows 8p+lo..8p+hi-1
                return sut[:, lo:hi, 0:W]

            def tdn(lo, hi):    # t(y+1) (plane 8 = boundary)
                return sut[:, lo + 1 : hi + 1, 0:W]

            def trt(lo, hi):    # t(x+1) (col W = -1 guard)
                return sut[:, lo:hi, 1 : W + 1]

            def qh_(lo, hi):    # qh rows lo..hi-1
                return qht[:, lo + 1 : hi + 1, :]

            def qhu(lo, hi):    # qh(y-1) (plane 0 = boundary)
                return qht[:, lo:hi, :]

            def sl(tile, lo, hi):
                return tile[:, lo:hi, :]

            # --- setup ---
            v.memset(sut[:, :, :], -1.0)
            v.memset(qht[:, :, :], 0.0)
            nc.sync.dma_start(out=o2t[:, :, :], in_=vu(o2_d))
            nc.sync.dma_start(out=g0t[:, :, :], in_=vu(g0_d))
            nc.sync.dma_start(out=g1t[:, :, :], in_=vu(g1_d))
            nc.sync.dma_start(out=gst[:, :, :], in_=vu(gs_d))
            nc.sync.dma_start(out=g1ut[:, :, :], in_=vu(g1u_d))

            def dma_ushift():
                # sut[p, 8, x] = t[8p+8, x] = sut[p+1, 0, x]; row 127 stays -1
                nc.sync.dma_start(
                    out=sut[0 : P - 1, NP, 0:W], in_=sut[1:P, 0, 0:W]
                )

            def dma_qshift():
                # qht[p, 0, x] = qh[8p-1, x] = qht[p-1, 8, x]; row 0 stays 0
                nc.sync.dma_start(
                    out=qht[1:P, 0, :], in_=qht[0 : P - 1, NP, :]
                )

            for _rep in range(reps):
                if reps > 1:
                    v.memset(sut[:, :, :], -1.0)
                    v.memset(qht[:, :, :], 0.0)
                for lo, hi in halves:
                    act.activation(t_(lo, hi), sl(o2t, lo, hi), AF.Tanh, scale=0.5)
                dma_ushift()

                for it in range(MAXITER):
                    last = it == MAXITER - 1
                    # --- products for the dual ---
                    # A = gs*t  (DVE planes 0..DV-1, GPSIMD planes DV..7)
                    v.tensor_mul(sl(tA, 0, DV), sl(gst, 0, DV), t_(0, DV))
                    gp.tensor_mul(sl(tA, DV, NP), sl(gst, DV, NP), t_(DV, NP))
                    # C~ = g0*t_rt (x+1 read is 2B-misaligned -> GPSIMD)
                    for lo, hi in halves:
                        gp.tensor_mul(sl(tC, lo, hi), sl(g0t, lo, hi), trt(lo, hi))
                    # B = g1*t_dn (plane-shifted view, aligned)
                    for lo, hi in halves:
                        v.tensor_mul(sl(tB, lo, hi), sl(g1t, lo, hi), tdn(lo, hi))
                    # --- dual chain: qh = relu(qh + A - B - C~) ---
                    for lo, hi in halves:
                        v.tensor_add(qh_(lo, hi), qh_(lo, hi), sl(tA, lo, hi))
                        v.tensor_sub(qh_(lo, hi), qh_(lo, hi), sl(tB, lo, hi))
                        v.tensor_sub(qh_(lo, hi), qh_(lo, hi), sl(tC, lo, hi))
                        act.activation(qh_(lo, hi), qh_(lo, hi), AF.Relu)
                    dma_qshift()
                    # --- products for the primal ---
                    # D = gs*qh (reuse tA), E = g1u*qh_u (tB), F = g0*qh (tC)
                    v.tensor_mul(sl(tA, 0, DV), sl(gst, 0, DV), qh_(0, DV))
                    gp.tensor_mul(sl(tA, DV, NP), sl(gst, DV, NP), qh_(DV, NP))
                    v.tensor_mul(sl(tB, 0, NP), sl(g1ut, 0, NP), qhu(0, NP))
                    for lo, hi in halves:
                        v.tensor_mul(sl(tC, lo, hi), sl(g0t, lo, hi), qh_(lo, hi))
                    # --- primal chain: s = o2 - D + E, s(x) += F(x-1) ---
                    for lo, hi in halves:
                        v.tensor_sub(sl(st, lo, hi), sl(o2t, lo, hi), sl(tA, lo, hi))
                        v.tensor_add(sl(st, lo, hi), sl(st, lo, hi), sl(tB, lo, hi))
                        gp.tensor_add(
                            st[:, lo:hi, 1:W], st[:, lo:hi, 1:W],
                            tC[:, lo:hi, 0 : W - 1],
                        )
                        if not last:
                            act.activation(
                                t_(lo, hi), sl(st, lo, hi), AF.Tanh, scale=0.5
                            )
                            if lo == 0:
                                dma_ushift()

            nc.sync.dma_start(out=vu(out_d), in_=st[:, :, :])

    nc.compile()
    return nc


def kernel(o, vector_field, nabla_w, div_w):
    global LAST_RESULTS
    if "nc" not in _CACHE:
        _CACHE["nc"] = _build()
    nc = _CACHE["nc"]

    o2 = (2.0 * np.asarray(o, dtype=np.float32)[:, 0]).astype(np.float16)
    vf = np.asarray(vector_field, dtype=np.float32)
    s = np.float32(1.0 / np.sqrt(2.0))
    g0f = vf[:, :, 0] * s
    g1f = vf[:, :, 1] * s
    g0 = g0f.astype(np.float16)
    g1 = g1f.astype(np.float16)
    gs = (g0f + g1f).astype(np.float16)
    g1uf = np.zeros_like(g1f)
    g1uf[1:] = g1f[:-1]
    g1u = g1uf.astype(np.float16)

    in_maps = [
        {"o2": np.ascontiguousarray(o2[b]), "g0": g0, "g1": g1, "gs": gs, "g1u": g1u}
        for b in range(B)
    ]
    res = bass_utils.run_bass_kernel_spmd(nc, in_maps, core_ids=list(range(B)))
    LAST_RESULTS = res
    return np.stack([r["out"] for r in res.results]).astype(np.float32)


# revision 7
# speedup vs baseline: 2.1730x; 1.2802x over previous
"""Trainium2 Bass kernel for the Chambolle-Pock-style primal/dual stencil loop.

Math (per image, H=W=1024, EPS=0.5, TAU=0.5, 10 iterations):
    u = sigmoid(o/EPS); q = 0
    repeat 10x:
        q  = relu(q - TAU*(vf1*Dy(u) + vf0*Dx(u)))   # forward diffs, zero pad
        Tq = BDy(vf1*q) + BDx(vf0*q)                  # backward diffs, zero pad
        u  = sigmoid((o - Tq)/EPS)
    return (o - Tq)/EPS

Rescaled (qh = 2*sqrt(2)*q, g = vf/sqrt(2), o2 = 2*o, t = tanh(s/2) with
s = 2(o - Tq); u-padding 0 becomes t-padding -1) and expanded so every
product has un-shifted operands where possible (gs = g0+g1, g1u = g1 shifted
down one row, host-precomputed):
    dual:   qh = relu(qh + gs*t - g1*t_dn - g0*t_rt)        # t_dn=t(y+1), t_rt=t(x+1)
    primal: s  = o2 - gs*qh + g1u*qh_u + (g0*qh)(x-1)       # qh_u=qh(y-1)
    t = tanh(s/2); output = s of the last iteration.

All state and arithmetic are fp16: the DVE runs tensor_tensor at 2 elem/cyc
for packed 2-byte operands, halving the per-pass cost vs fp32.  fp16 (not
bf16) because the relu makes isolated pixels chaotic under per-step rounding
noise: measured rel-L2 vs the fp32 reference is ~8e-3 for fp16 end-to-end but
~3e-2 for bf16 (over the 2e-2 gate).  Value ranges (|qh|<12, |s|<32) are far
from fp16 overflow.

Engine split: the two x-shifted accesses (reading t(x+1) for the C~ product
and (g0*qh)(x-1) for the F accumulation) have 2-byte-misaligned operands,
which breaks the DVE 2x perf mode's 4B-alignment requirement on real HW.
So: GPSIMD (alignment-indifferent) computes the off-chain products
C~ = g0*t(x+1) and F' = g0*qh; a DMA engine produces the column-shifted
copy F'(x-1) (byte-granular, runs on otherwise-idle DMA queues); and the
DVE runs every in-chain accumulation plus the A/B/D/E products, all
4B-aligned at 2 elem/cycle.  relu/tanh run on the Scalar(Act) engine.
Everything operates at quarter (2-plane) granularity so the four engine
streams pipeline within and across iterations: per iteration the DVE does
~80 plane-passes at 0.55us, GPSIMD 16 at 2.05us, Act 16 at 0.85us.

Sharding: pure data parallel, one image per NeuronCore (B=8 over 8 cores),
g-fields broadcast.  Layout: image row y = 8*p + i -> partition p (0..127),
plane i (0..7); +-1 row shifts are free-dim plane offsets with one guard
plane filled by a tiny SBUF->SBUF DMA per iteration; x+1 uses a guard column.
Whole working set (~180KB/partition) is SBUF resident: HBM traffic is one
10MB fp16 load + 2MB store per core.
"""

import numpy as np

import concourse.bacc as bacc
import concourse.mybir as mybir
from concourse.tile import TileContext
from concourse import bass_utils

F16 = mybir.dt.float16
AF = mybir.ActivationFunctionType
ALU = mybir.AluOpType

B, H, W = 8, 1024, 1024
P = 128          # SBUF partitions
NP = H // P      # planes per partition = 8
WG = W + 2       # t-plane width incl. guard column (even, keeps 4B align)
MAXITER = 10
NQ = 4           # quarters (2 planes each) for pipelining

_CACHE = {}
LAST_RESULTS = None  # BassKernelResults of the most recent run (for test.py)


def _build(reps=1):
    """Build the Bass program.  reps>1 repeats the whole computation (state
    re-initialized each rep, same output) — used only for wall-clock timing
    of the HW kernel when no NTFF profiling is available."""
    nc = bacc.Bacc("TRN2", target_bir_lowering=False, debug=False)

    o2_d = nc.dram_tensor("o2", [H, W], F16, kind="ExternalInput").ap()
    g0_d = nc.dram_tensor("g0", [H, W], F16, kind="ExternalInput").ap()
    g1_d = nc.dram_tensor("g1", [H, W], F16, kind="ExternalInput").ap()
    gs_d = nc.dram_tensor("gs", [H, W], F16, kind="ExternalInput").ap()
    g1u_d = nc.dram_tensor("g1u", [H, W], F16, kind="ExternalInput").ap()
    out_d = nc.dram_tensor("out", [H, W], F16, kind="ExternalOutput").ap()

    # (H, W) -> (p, i, x) with y = 8*p + i
    def vu(ap):
        return ap.rearrange("(p i) x -> p i x", i=NP)

    v = nc.vector
    gp = nc.gpsimd
    act = nc.scalar

    with TileContext(nc) as tc:
        with tc.tile_pool(name="main", bufs=1) as pool:
            o2t = pool.tile([P, NP, W], F16)
            g0t = pool.tile([P, NP, W], F16)
            g1t = pool.tile([P, NP, W], F16)
            gst = pool.tile([P, NP, W], F16)
            g1ut = pool.tile([P, NP, W], F16)
            st = pool.tile([P, NP, W], F16)
            # t-state: planes 0..7 = t at cols 0..W-1, col W = -1 guard for
            # x+1 reads, plane 8 = boundary row t[8p+8] (partition 127: -1)
            sut = pool.tile([P, NP + 1, WG], F16)
            # qh-state: planes 1..8 = qh rows 8p..8p+7, plane 0 = boundary
            # row qh[8p-1] (partition 0: zero pad)
            qht = pool.tile([P, NP + 1, W], F16)
            tA = pool.tile([P, NP, W], F16)
            tB = pool.tile([P, NP, W], F16)
            tC = pool.tile([P, NP, W], F16)
            # F'(x-1): col 0 is a permanent zero guard, cols 1..W-1 DMA'd
            tC2 = pool.tile([P, NP, W], F16)

            quarters = [(2 * q, 2 * q + 2) for q in range(NQ)]

            def t_(lo, hi):     # t rows 8p+lo..8p+hi-1
                return sut[:, lo:hi, 0:W]

            def tdn(lo, hi):    # t(y+1) (plane 8 = boundary)
                return sut[:, lo + 1 : hi + 1, 0:W]

            def trt(lo, hi):    # t(x+1) (col W = -1 guard)
                return sut[:, lo:hi, 1 : W + 1]

            def qh_(lo, hi):    # qh rows lo..hi-1
                return qht[:, lo + 1 : hi + 1, :]

            def qhu(lo, hi):    # qh(y-1) (plane 0 = boundary)
                return qht[:, lo:hi, :]

            def sl(tile, lo, hi):
                return tile[:, lo:hi, :]

            # --- setup ---
            v.memset(sut[:, :, :], -1.0)
            v.memset(qht[:, :, :], 0.0)
            v.memset(tC2[:, :, :], 0.0)
            nc.sync.dma_start(out=o2t[:, :, :], in_=vu(o2_d))
            nc.sync.dma_start(out=g0t[:, :, :], in_=vu(g0_d))
            nc.sync.dma_start(out=g1t[:, :, :], in_=vu(g1_d))
            nc.sync.dma_start(out=gst[:, :, :], in_=vu(gs_d))
            nc.sync.dma_start(out=g1ut[:, :, :], in_=vu(g1u_d))

            def dma_ushift():
                # sut[p, 8, x] = t[8p+8, x] = sut[p+1, 0, x]; row 127 stays -1
                nc.sync.dma_start(
                    out=sut[0 : P - 1, NP, 0:W], in_=sut[1:P, 0, 0:W]
                )

            def dma_qshift():
                # qht[p, 0, x] = qh[8p-1, x] = qht[p-1, 8, x]; row 0 stays 0
                nc.sync.dma_start(
                    out=qht[1:P, 0, :], in_=qht[0 : P - 1, NP, :]
                )

            def dma_fshift(lo, hi):
                # tC2[y, x] = F'[y, x-1]; col 0 stays 0
                nc.sync.dma_start(
                    out=tC2[:, lo:hi, 1:W], in_=tC[:, lo:hi, 0 : W - 1]
                )

            for _rep in range(reps):
                if reps > 1:
                    v.memset(sut[:, :, :], -1.0)
                    v.memset(qht[:, :, :], 0.0)
                for lo, hi in quarters:
                    act.activation(t_(lo, hi), sl(o2t, lo, hi), AF.Tanh, scale=0.5)
                    if lo == 0:
                        dma_ushift()

                for it in range(MAXITER):
                    last = it == MAXITER - 1
                    # --- dual products (off-chain) ---
                    # GPSIMD: C~ = g0*t(x+1)  (2B-misaligned read is free here)
                    for lo, hi in quarters:
                        gp.tensor_mul(sl(tC, lo, hi), sl(g0t, lo, hi), trt(lo, hi))
                    # DVE: A = gs*t, B = g1*t(y+1)
                    for lo, hi in quarters:
                        v.tensor_mul(sl(tA, lo, hi), sl(gst, lo, hi), t_(lo, hi))
                        v.tensor_mul(sl(tB, lo, hi), sl(g1t, lo, hi), tdn(lo, hi))
                    # --- dual chain: qh = relu(qh + A - B - C~) ---
                    for lo, hi in quarters:
                        v.tensor_add(qh_(lo, hi), qh_(lo, hi), sl(tA, lo, hi))
                        v.tensor_sub(qh_(lo, hi), qh_(lo, hi), sl(tB, lo, hi))
                        v.tensor_sub(qh_(lo, hi), qh_(lo, hi), sl(tC, lo, hi))
                        act.activation(qh_(lo, hi), qh_(lo, hi), AF.Relu)
                    dma_qshift()
                    # --- primal products ---
                    # GPSIMD: F' = g0*qh (then a DMA engine shifts it right by
                    # one column into tC2, so the s-accumulation is aligned)
                    for lo, hi in quarters:
                        gp.tensor_mul(sl(tC, lo, hi), sl(g0t, lo, hi), qh_(lo, hi))
                        dma_fshift(lo, hi)
                    # DVE: D = gs*qh (reuse tA); E = g1u*qh(y-1) (tB), split so
                    # each piece depends on one relu quarter (rows 0 and 7 via
                    # the qDMA boundary / last relu quarter)
                    v.tensor_mul(sl(tA, 0, 2), sl(gst, 0, 2), qh_(0, 2))
                    for a in range(3):
                        v.tensor_mul(
                            sl(tB, 2 * a + 1, 2 * a + 3),
                            sl(g1ut, 2 * a + 1, 2 * a + 3),
                            qhu(2 * a + 1, 2 * a + 3),
                        )
                        v.tensor_mul(
                            sl(tA, 2 * a + 2, 2 * a + 4),
                            sl(gst, 2 * a + 2, 2 * a + 4),
                            qh_(2 * a + 2, 2 * a + 4),
                        )
                    v.tensor_mul(sl(tB, 7, 8), sl(g1ut, 7, 8), qhu(7, 8))
                    v.tensor_mul(sl(tB, 0, 1), sl(g1ut, 0, 1), qhu(0, 1))
                    # --- primal chain: s = o2 - D + E + F'(x-1) ---
                    for lo, hi in quarters:
                        v.tensor_sub(sl(st, lo, hi), sl(o2t, lo, hi), sl(tA, lo, hi))
                        v.tensor_add(sl(st, lo, hi), sl(st, lo, hi), sl(tB, lo, hi))
                        v.tensor_add(sl(st, lo, hi), sl(st, lo, hi), sl(tC2, lo, hi))
                        if not last:
                            act.activation(
                                t_(lo, hi), sl(st, lo, hi), AF.Tanh, scale=0.5
                            )
                            if lo == 0:
                                dma_ushift()

            nc.sync.dma_start(out=vu(out_d), in_=st[:, :, :])

    nc.compile()
    return nc


def kernel(o, vector_field, nabla_w, div_w):
    global LAST_RESULTS
    if "nc" not in _CACHE:
        _CACHE["nc"] = _build()
    nc = _CACHE["nc"]

    o2 = (2.0 * np.asarray(o, dtype=np.float32)[:, 0]).astype(np.float16)
    vf = np.asarray(vector_field, dtype=np.float32)
    s = np.float32(1.0 / np.sqrt(2.0))
    g0f = vf[:, :, 0] * s
    g1f = vf[:, :, 1] * s
    g0 = g0f.astype(np.float16)
    g1 = g1f.astype(np.float16)
    gs = (g0f + g1f).astype(np.float16)
    g1uf = np.zeros_like(g1f)
    g1uf[1:] = g1f[:-1]
    g1u = g1uf.astype(np.float16)

    in_maps = [
        {"o2": np.ascontiguousarray(o2[b]), "g0": g0, "g1": g1, "gs": gs, "g1u": g1u}
        for b in range(B)
    ]
    res = bass_utils.run_bass_kernel_spmd(nc, in_maps, core_ids=list(range(B)))
    LAST_RESULTS = res
    return np.stack([r["out"] for r in res.results]).astype(np.float32)


# revision 10
# speedup vs baseline: 2.3498x; 1.0814x over previous
"""Trainium2 Bass kernel for the Chambolle-Pock-style primal/dual stencil loop.

Math (per image, H=W=1024, EPS=0.5, TAU=0.5, 10 iterations):
    u = sigmoid(o/EPS); q = 0
    repeat 10x:
        q  = relu(q - TAU*(vf1*Dy(u) + vf0*Dx(u)))   # forward diffs, zero pad
        Tq = BDy(vf1*q) + BDx(vf0*q)                  # backward diffs, zero pad
        u  = sigmoid((o - Tq)/EPS)
    return (o - Tq)/EPS

Rescaled (qh = 2*sqrt(2)*q, g = vf/sqrt(2), o2 = 2*o, t = tanh(s/2) with
s = 2(o - Tq); u-padding 0 becomes t-padding -1) and with every y-shifted
product rewritten through a host-preshifted field (gs = g0+g1, g1d(y) =
g1(y-1)) so each product depends on exactly one tanh/relu quarter:
    K = g1d*t; H = g1*qh                  # then B(y)=g1(y)*t(y+1) = K(y+1)
    dual:   qh = relu(qh + gs*t - K(y+1) - g0*t(x+1))
    primal: s  = o2 - gs*qh + H(y-1) + (g0*qh)(x-1)
    t = tanh(s/2); output = s of the last iteration.

All state and arithmetic are fp16: the DVE runs tensor_tensor at 2 elem/cyc
for packed 2-byte operands, halving the per-pass cost vs fp32.  fp16 (not
bf16) because the relu makes isolated pixels chaotic under per-step rounding
noise: measured rel-L2 vs the fp32 reference is ~1e-2 for fp16 end-to-end but
~3e-2 for bf16 (over the 2e-2 gate).  Value ranges (|qh|<12, |s|<32) are far
from fp16 overflow.

Engine split: the two x-shifted accesses (reading t(x+1) for the C~ product
and (g0*qh)(x-1) for the F accumulation) have 2-byte-misaligned operands,
which breaks the DVE 2x perf mode's 4B-alignment requirement on real HW.
So: GPSIMD (alignment-indifferent) computes the off-chain products
C~ = g0*t(x+1) and F' = g0*qh; a DMA engine produces the column-shifted
copy F'(x-1) (byte-granular, runs on otherwise-idle DMA queues); and the
DVE runs every in-chain accumulation plus the A/K/H/D products, all
4B-aligned at 2 elem/cycle.  relu/tanh run on the Scalar(Act) engine.

Layout: image row y = 8*p + i -> partition p (0..127), plane i (0..7).
Everything operates at quarter (2-plane) granularity in fixed order
(1,2,3,0) so the four engine streams pipeline within and across iterations;
the K/H plane-8/plane-0 boundary rows move between partitions via tiny
SBUF->SBUF DMAs whose consumers sit half an iteration away (no stalls).
Per iteration the DVE does 80 plane-passes at ~0.56us, GPSIMD 16 at 2.05us,
Act 16 at ~0.85us.  The whole working set (~196KB/partition) is SBUF
resident: HBM traffic is one 10MB fp16 load + 2MB store per core.

Sharding: pure data parallel, one image per NeuronCore (B=8 over 8 cores),
g-fields broadcast.
"""

import numpy as np

import concourse.bacc as bacc
import concourse.mybir as mybir
from concourse.tile import TileContext
from concourse import bass_utils

F16 = mybir.dt.float16
AF = mybir.ActivationFunctionType

B, H, W = 8, 1024, 1024
P = 128          # SBUF partitions
NP = H // P      # planes per partition = 8
WG = W + 2       # t-plane width incl. guard column (even, keeps 4B align)
MAXITER = 10
QORD = ((2, 4), (4, 6), (6, 8), (0, 2))   # quarter order 1,2,3,0

_CACHE = {}
LAST_RESULTS = None  # BassKernelResults of the most recent run (for test.py)


def _build(reps=1):
    """Build the Bass program.  reps>1 repeats the whole computation (state
    re-initialized each rep, same output) — used only for wall-clock timing
    of the HW kernel when no NTFF profiling is available."""
    nc = bacc.Bacc("TRN2", target_bir_lowering=False, debug=False)

    o2_d = nc.dram_tensor("o2", [H, W], F16, kind="ExternalInput").ap()
    g0_d = nc.dram_tensor("g0", [H, W], F16, kind="ExternalInput").ap()
    g1_d = nc.dram_tensor("g1", [H, W], F16, kind="ExternalInput").ap()
    gs_d = nc.dram_tensor("gs", [H, W], F16, kind="ExternalInput").ap()
    g1d_d = nc.dram_tensor("g1d", [H, W], F16, kind="ExternalInput").ap()
    km_d = nc.dram_tensor("km", [1, W], F16, kind="ExternalInput").ap()
    out_d = nc.dram_tensor("out", [H, W], F16, kind="ExternalOutput").ap()

    # (H, W) -> (p, i, x) with y = 8*p + i
    def vu(ap):
        return ap.rearrange("(p i) x -> p i x", i=NP)

    v = nc.vector
    gp = nc.gpsimd
    act = nc.scalar

    with TileContext(nc) as tc:
        with tc.tile_pool(name="main", bufs=1) as pool:
            o2t = pool.tile([P, NP, W], F16)
            g0t = pool.tile([P, NP, W], F16)
            g1t = pool.tile([P, NP, W], F16)
            gst = pool.tile([P, NP, W], F16)
            g1dt = pool.tile([P, NP, W], F16)
            # t / s state: planes 0..7, col W = -1 guard for x+1 reads
            sut = pool.tile([P, NP, WG], F16)
            qht = pool.tile([P, NP, W], F16)
            tA = pool.tile([P, NP, W], F16)
            # K = g1d*t at planes 0..7; plane 8 = K[8p+8] boundary
            # (partition 127: constant -g1[1023] = K at the t=-1 pad row)
            tK = pool.tile([P, NP + 1, W], F16)
            # H = g1*qh at planes 1..8; plane 0 = H[8p-1] boundary
            # (partition 0: zero pad)
            tH = pool.tile([P, NP + 1, W], F16)
            tC = pool.tile([P, NP, W], F16)
            # F'(x-1): col 0 is a permanent zero guard, cols 1..W-1 DMA'd
            tC2 = pool.tile([P, NP, W], F16)

            def t_(lo, hi):     # t rows 8p+lo..8p+hi-1
                return sut[:, lo:hi, 0:W]

            def trt(lo, hi):    # t(x+1) (col W = -1 guard)
                return sut[:, lo:hi, 1 : W + 1]

            def bv(lo, hi):     # B(y) = K(y+1) (plane 8 = boundary)
                return tK[:, lo + 1 : hi + 1, :]

            def ev(lo, hi):     # E(y) = H(y-1) (plane 0 = boundary)
                return tH[:, lo:hi, :]

            def qh_(lo, hi):
                return qht[:, lo:hi, :]

            def sl(tile, lo, hi):
                return tile[:, lo:hi, :]

            # --- setup ---
            # Only guard regions need init: everything else is written
            # before its first read.
            v.memset(sut[:, :, W:WG], -1.0)        # x+1 guard column = -1
            v.memset(tH[0:1, 0, :], 0.0)           # H[-1] pad row = 0
            v.memset(tC2[:, :, 0:1], 0.0)          # F'(x-1) zero at x=0
            nc.sync.dma_start(out=tK[P - 1 : P, NP, :], in_=km_d)
            o2q = vu(o2_d)
            for lo, hi in QORD:
                nc.sync.dma_start(out=o2t[:, lo:hi, :], in_=o2q[:, lo:hi, :])
            nc.sync.dma_start(out=gst[:, :, :], in_=vu(gs_d))
            nc.sync.dma_start(out=g1dt[:, :, :], in_=vu(g1d_d))
            nc.sync.dma_start(out=g0t[:, :, :], in_=vu(g0_d))
            nc.sync.dma_start(out=g1t[:, :, :], in_=vu(g1_d))

            def dma_kshift():
                # tK[p, 8] = K[8p+8] = tK[p+1, 0]; partition 127 keeps km
                nc.sync.dma_start(
                    out=tK[0 : P - 1, NP, :], in_=tK[1:P, 0, :]
                )

            def dma_hshift():
                # tH[p, 0] = H[8p-1] = tH[p-1, 8]; partition 0 stays 0
                nc.sync.dma_start(
                    out=tH[1:P, 0, :], in_=tH[0 : P - 1, NP, :]
                )

            def dma_fshift(lo, hi):
                # tC2[y, x] = F'[y, x-1]; col 0 stays 0
                nc.sync.dma_start(
                    out=tC2[:, lo:hi, 1:W], in_=tC[:, lo:hi, 0 : W - 1]
                )

            for _rep in range(reps):
                for lo, hi in QORD:
                    act.activation(t_(lo, hi), sl(o2t, lo, hi), AF.Tanh, scale=0.5)

                for it in range(MAXITER):
                    first = it == 0
                    last = it == MAXITER - 1
                    # --- dual products (each needs one tanh quarter) ---
                    # GPSIMD: C~ = g0*t(x+1)  (2B-misaligned read is free)
                    for lo, hi in QORD:
                        gp.tensor_mul(sl(tC, lo, hi), sl(g0t, lo, hi), trt(lo, hi))
                    # DVE: A = gs*t, K = g1d*t
                    for lo, hi in QORD:
                        v.tensor_mul(sl(tA, lo, hi), sl(gst, lo, hi), t_(lo, hi))
                        v.tensor_mul(sl(tK, lo, hi), sl(g1dt, lo, hi), t_(lo, hi))
                    dma_kshift()
                    # --- dual chain: qh = relu(qh + A - K(y+1) - C~) ---
                    for lo, hi in QORD:
                        if first:
                            v.tensor_sub(qh_(lo, hi), sl(tA, lo, hi), bv(lo, hi))
                        else:
                            v.tensor_add(qh_(lo, hi), qh_(lo, hi), sl(tA, lo, hi))
                            v.tensor_sub(qh_(lo, hi), qh_(lo, hi), bv(lo, hi))
                        v.tensor_sub(qh_(lo, hi), qh_(lo, hi), sl(tC, lo, hi))
                        act.activation(qh_(lo, hi), qh_(lo, hi), AF.Relu)
                    # --- primal products (each needs one relu quarter) ---
                    # GPSIMD: F' = g0*qh -> DMA shifts it right one column
                    for lo, hi in QORD:
                        gp.tensor_mul(sl(tC, lo, hi), sl(g0t, lo, hi), qh_(lo, hi))
                        dma_fshift(lo, hi)
                    # DVE: H = g1*qh (planes 1..8 of tH), D = gs*qh (tA)
                    for i, (lo, hi) in enumerate(QORD):
                        v.tensor_mul(
                            tH[:, lo + 1 : hi + 1, :], sl(g1t, lo, hi), qh_(lo, hi)
                        )
                        if (lo, hi) == (6, 8):
                            dma_hshift()
                        v.tensor_mul(sl(tA, lo, hi), sl(gst, lo, hi), qh_(lo, hi))
                    # --- primal chain: s = o2 - D + H(y-1) + F'(x-1) ---
                    # (s overwrites t in sut; tanh then maps it back to t)
                    for lo, hi in QORD:
                        v.tensor_sub(t_(lo, hi), sl(o2t, lo, hi), sl(tA, lo, hi))
                        v.tensor_add(t_(lo, hi), t_(lo, hi), ev(lo, hi))
                        v.tensor_add(t_(lo, hi), t_(lo, hi), sl(tC2, lo, hi))
                        if not last:
                            act.activation(
                                t_(lo, hi), t_(lo, hi), AF.Tanh, scale=0.5
                            )
                        else:
                            nc.sync.dma_start(
                                out=vu(out_d)[:, lo:hi, :], in_=t_(lo, hi)
                            )

    nc.compile()
    return nc


def kernel(o, vector_field, nabla_w, div_w):
    global LAST_RESULTS
    if "nc" not in _CACHE:
        _CACHE["nc"] = _build()
    nc = _CACHE["nc"]

    o2 = (2.0 * np.asarray(o, dtype=np.float32)[:, 0]).astype(np.float16)
    vf = np.asarray(vector_field, dtype=np.float32)
    s = np.float32(1.0 / np.sqrt(2.0))
    g0f = vf[:, :, 0] * s
    g1f = vf[:, :, 1] * s
    g0 = g0f.astype(np.float16)
    g1 = g1f.astype(np.float16)
    gs = (g0f + g1f).astype(np.float16)
    g1df = np.zeros_like(g1f)
    g1df[1:] = g1f[:-1]
    g1d = g1df.astype(np.float16)
    km = np.ascontiguousarray(-g1[1023:1024, :])  # K at the t(1024)=-1 pad row

    in_maps = [
        {
            "o2": np.ascontiguousarray(o2[b]),
            "g0": g0,
            "g1": g1,
            "gs": gs,
            "g1d": g1d,
            "km": km,
        }
        for b in range(B)
    ]
    res = bass_utils.run_bass_kernel_spmd(nc, in_maps, core_ids=list(range(B)))
    LAST_RESULTS = res
    return np.stack([r["out"] for r in res.results]).astype(np.float32)


# revision 21
# speedup vs baseline: 2.4053x; 1.0236x over previous
"""Trainium2 Bass kernel for the Chambolle-Pock-style primal/dual stencil loop.

Math (per image, H=W=1024, EPS=0.5, TAU=0.5, 10 iterations):
    u = sigmoid(o/EPS); q = 0
    repeat 10x:
        q  = relu(q - TAU*(vf1*Dy(u) + vf0*Dx(u)))   # forward diffs, zero pad
        Tq = BDy(vf1*q) + BDx(vf0*q)                  # backward diffs, zero pad
        u  = sigmoid((o - Tq)/EPS)
    return (o - Tq)/EPS

Rescaled (qh = 2*sqrt(2)*q, g = vf/sqrt(2), o2 = 2*o, t = tanh(s/2) with
s = 2(o - Tq); u-padding 0 becomes t-padding -1) and with every y-shifted
product rewritten through a host-preshifted field (gs = g0+g1, g1d(y) =
g1(y-1)) so each product depends on exactly one tanh/relu quarter:
    K = g1d*t; H = g1*qh                  # then B(y)=g1(y)*t(y+1) = K(y+1)
    dual:   qh = relu(qh + gs*t - K(y+1) - g0*t(x+1))
    primal: s  = o2 - gs*qh + H(y-1) + (g0*qh)(x-1)
    t = tanh(s/2); output = s of the last iteration.

All state and arithmetic are fp16: the DVE runs tensor_tensor at 2 elem/cyc
for packed 2-byte operands, halving the per-pass cost vs fp32.  fp16 (not
bf16) because the relu makes isolated pixels chaotic under per-step rounding
noise: measured rel-L2 vs the fp32 reference is ~1e-2 for fp16 end-to-end but
~3e-2 for bf16 (over the 2e-2 gate).  Value ranges (|qh|<12, |s|<32) are far
from fp16 overflow.

Engine split: the two x-shifted accesses (reading t(x+1) for the C~ product
and (g0*qh)(x-1) for the F accumulation) have 2-byte-misaligned operands,
which breaks the DVE 2x perf mode's 4B-alignment requirement on real HW.
So: GPSIMD (alignment-indifferent) computes the off-chain products
C~ = g0*t(x+1) and F' = g0*qh; a DMA engine produces the column-shifted
copy F'(x-1) (byte-granular, runs on otherwise-idle DMA queues); and the
DVE runs every in-chain accumulation plus the A/K/H/D products, all
4B-aligned at 2 elem/cycle.  relu/tanh run on the Scalar(Act) engine.

Layout: image row y = 8*p + i -> partition p (0..127), plane i (0..7).
Everything operates at quarter (2-plane) granularity in fixed order
(1,2,3,0) so the four engine streams pipeline within and across iterations;
the K/H plane-8/plane-0 boundary rows move between partitions via tiny
SBUF->SBUF DMAs whose consumers sit half an iteration away (no stalls).
Per iteration the DVE does 80 plane-passes at ~0.56us, GPSIMD 16 at 2.05us,
Act 16 at ~0.85us.  The whole working set (~196KB/partition) is SBUF
resident: HBM traffic is one 10MB fp16 load + 2MB store per core.

Sharding: pure data parallel, one image per NeuronCore (B=8 over 8 cores),
g-fields broadcast.
"""

import numpy as np

import concourse.bacc as bacc
import concourse.mybir as mybir
from concourse.tile import TileContext
from concourse import bass_utils

F16 = mybir.dt.float16
AF = mybir.ActivationFunctionType

B, H, W = 8, 1024, 1024
P = 128          # SBUF partitions
NP = H // P      # planes per partition = 8
WG = W + 2       # t-plane width incl. guard column (even, keeps 4B align)
MAXITER = 10
QORD = ((2, 4), (4, 6), (6, 8), (0, 2))   # quarter order 1,2,3,0

# Single-plane product offloads DVE -> GPSIMD for load balance.  Keys name
# the product and quarter; each moves one 1024-elem plane-pass.
OFFLOAD = {"A0": True, "K0": True, "D2": True, "D3": False, "H2": True}

_CACHE = {}
LAST_RESULTS = None  # BassKernelResults of the most recent run (for test.py)


def _build(reps=1):
    """Build the Bass program.  reps>1 repeats the whole computation (state
    re-initialized each rep, same output) — used only for wall-clock timing
    of the HW kernel when no NTFF profiling is available."""
    nc = bacc.Bacc("TRN2", target_bir_lowering=False, debug=False)

    o2_d = nc.dram_tensor("o2", [H, W], F16, kind="ExternalInput").ap()
    g0_d = nc.dram_tensor("g0", [H, W], F16, kind="ExternalInput").ap()
    gs_d = nc.dram_tensor("gs", [H, W], F16, kind="ExternalInput").ap()
    g1d_d = nc.dram_tensor("g1d", [H, W], F16, kind="ExternalInput").ap()
    km_d = nc.dram_tensor("km", [1, W], F16, kind="ExternalInput").ap()
    out_d = nc.dram_tensor("out", [H, W], F16, kind="ExternalOutput").ap()

    # (H, W) -> (p, i, x) with y = 8*p + i
    def vu(ap):
        return ap.rearrange("(p i) x -> p i x", i=NP)

    v = nc.vector
    gp = nc.gpsimd
    act = nc.scalar

    with TileContext(nc) as tc:
        with tc.tile_pool(name="main", bufs=1) as pool:
            o2t = pool.tile([P, NP, W], F16)
            g0t = pool.tile([P, NP, W], F16)
            g1t = pool.tile([P, NP, W], F16)
            gst = pool.tile([P, NP, W], F16)
            g1dt = pool.tile([P, NP, W], F16)
            # t / s state: planes 0..7, col W = -1 guard for x+1 reads
            sut = pool.tile([P, NP, WG], F16)
            qht = pool.tile([P, NP, W], F16)
            tA = pool.tile([P, NP, W], F16)
            # K = g1d*t at planes 0..7; plane 8 = K[8p+8] boundary
            # (partition 127: constant -g1[1023] = K at the t=-1 pad row)
            tK = pool.tile([P, NP + 1, W], F16)
            # H = g1*qh at planes 1..8; plane 0 = H[8p-1] boundary
            # (partition 0: zero pad)
            tH = pool.tile([P, NP + 1, W], F16)
            tC = pool.tile([P, NP, W], F16)
            # F'(x-1): col 0 is a permanent zero guard, cols 1..W-1 DMA'd
            tC2 = pool.tile([P, NP, W], F16)

            def t_(lo, hi):     # t rows 8p+lo..8p+hi-1
                return sut[:, lo:hi, 0:W]

            def trt(lo, hi):    # t(x+1) (col W = -1 guard)
                return sut[:, lo:hi, 1 : W + 1]

            def bv(lo, hi):     # B(y) = K(y+1) (plane 8 = boundary)
                return tK[:, lo + 1 : hi + 1, :]

            def ev(lo, hi):     # E(y) = H(y-1) (plane 0 = boundary)
                return tH[:, lo:hi, :]

            def qh_(lo, hi):
                return qht[:, lo:hi, :]

            def sl(tile, lo, hi):
                return tile[:, lo:hi, :]

            # --- setup ---
            # Only guard regions need init: everything else is written
            # before its first read.
            v.memset(sut[:, :, W:WG], -1.0)        # x+1 guard column = -1
            v.memset(tH[0:1, 0, :], 0.0)           # H[-1] pad row = 0
            v.memset(tC2[:, :, 0:1], 0.0)          # F'(x-1) zero at x=0
            # Loads are HBM-bandwidth serial (~1.6us per field quarter), so
            # interleave them in consumer need order.  g1 itself is derived
            # on-chip (g1 = gs - g0, in iteration 0) to cut the load volume.
            nc.sync.dma_start(out=tK[P - 1 : P, NP, :], in_=km_d)
            for lo, hi in QORD:
                nc.sync.dma_start(
                    out=o2t[:, lo:hi, :], in_=vu(o2_d)[:, lo:hi, :]
                )
                nc.sync.dma_start(
                    out=g0t[:, lo:hi, :], in_=vu(g0_d)[:, lo:hi, :]
                )
                nc.sync.dma_start(
                    out=gst[:, lo:hi, :], in_=vu(gs_d)[:, lo:hi, :]
                )
                nc.sync.dma_start(
                    out=g1dt[:, lo:hi, :], in_=vu(g1d_d)[:, lo:hi, :]
                )

            def dma_kshift():
                # tK[p, 8] = K[8p+8] = tK[p+1, 0]; partition 127 keeps km
                nc.sync.dma_start(
                    out=tK[0 : P - 1, NP, :], in_=tK[1:P, 0, :]
                )

            def dma_hshift():
                # tH[p, 0] = H[8p-1] = tH[p-1, 8]; partition 0 stays 0
                nc.sync.dma_start(
                    out=tH[1:P, 0, :], in_=tH[0 : P - 1, NP, :]
                )

            def dma_fshift(lo, hi):
                # tC2[y, x] = F'[y, x-1]; col 0 stays 0
                nc.sync.dma_start(
                    out=tC2[:, lo:hi, 1:W], in_=tC[:, lo:hi, 0 : W - 1]
                )

            for _rep in range(reps):
                for lo, hi in QORD:
                    act.activation(t_(lo, hi), sl(o2t, lo, hi), AF.Tanh, scale=0.5)

                for it in range(MAXITER):
                    first = it == 0
                    last = it == MAXITER - 1
                    # --- dual products (each needs one tanh quarter) ---
                    # GPSIMD: q0 plane-0 shares first (input ready at the
                    # iteration boundary), then C~ = g0*t(x+1) per quarter
                    # (the 2B-misaligned read is free on GPSIMD)
                    if OFFLOAD["A0"]:
                        gp.tensor_mul(sl(tA, 0, 1), sl(gst, 0, 1), t_(0, 1))
                    if OFFLOAD["K0"]:
                        gp.tensor_mul(sl(tK, 0, 1), sl(g1dt, 0, 1), t_(0, 1))
                    for lo, hi in QORD:
                        gp.tensor_mul(sl(tC, lo, hi), sl(g0t, lo, hi), trt(lo, hi))
                    # DVE: A = gs*t, K = g1d*t
                    for lo, hi in QORD:
                        alo = lo + 1 if lo == 0 and OFFLOAD["A0"] else lo
                        klo = lo + 1 if lo == 0 and OFFLOAD["K0"] else lo
                        v.tensor_mul(sl(tA, alo, hi), sl(gst, alo, hi), t_(alo, hi))
                        v.tensor_mul(sl(tK, klo, hi), sl(g1dt, klo, hi), t_(klo, hi))
                        if first and lo != 0:
                            v.tensor_sub(
                                sl(g1t, lo, hi), sl(gst, lo, hi), sl(g0t, lo, hi)
                            )
                    dma_kshift()
                    # --- dual chain: qh = relu(qh + A - K(y+1) - C~) ---
                    for lo, hi in QORD:
                        if first:
                            v.tensor_sub(qh_(lo, hi), sl(tA, lo, hi), bv(lo, hi))
                        else:
                            v.tensor_add(qh_(lo, hi), qh_(lo, hi), sl(tA, lo, hi))
                            v.tensor_sub(qh_(lo, hi), qh_(lo, hi), bv(lo, hi))
                        v.tensor_sub(qh_(lo, hi), qh_(lo, hi), sl(tC, lo, hi))
                        act.activation(qh_(lo, hi), qh_(lo, hi), AF.Relu)
                    # --- primal products (each needs one relu quarter) ---
                    # GPSIMD: F' = g0*qh -> DMA shifts it right one column;
                    # DVE: H = g1*qh (planes 1..8 of tH), D = gs*qh (tA);
                    # mid-iteration plane shares go to GPSIMD
                    if first:
                        v.tensor_sub(sl(g1t, 0, 2), sl(gst, 0, 2), sl(g0t, 0, 2))
                    for lo, hi in QORD:
                        gp.tensor_mul(sl(tC, lo, hi), sl(g0t, lo, hi), qh_(lo, hi))
                        dma_fshift(lo, hi)
                        if (lo, hi) == (4, 6) and OFFLOAD["H2"]:
                            gp.tensor_mul(tH[:, 5:6, :], sl(g1t, 4, 5), qh_(4, 5))
                            v.tensor_mul(tH[:, 6:7, :], sl(g1t, 5, 6), qh_(5, 6))
                        else:
                            v.tensor_mul(
                                tH[:, lo + 1 : hi + 1, :], sl(g1t, lo, hi),
                                qh_(lo, hi),
                            )
                        if (lo, hi) == (6, 8):
                            dma_hshift()
                        if (lo, hi) == (4, 6) and OFFLOAD["D2"]:
                            gp.tensor_mul(sl(tA, 4, 5), sl(gst, 4, 5), qh_(4, 5))
                            v.tensor_mul(sl(tA, 5, 6), sl(gst, 5, 6), qh_(5, 6))
                        elif (lo, hi) == (6, 8) and OFFLOAD["D3"]:
                            gp.tensor_mul(sl(tA, 6, 7), sl(gst, 6, 7), qh_(6, 7))
                            v.tensor_mul(sl(tA, 7, 8), sl(gst, 7, 8), qh_(7, 8))
                        else:
                            v.tensor_mul(sl(tA, lo, hi), sl(gst, lo, hi), qh_(lo, hi))
                    # --- primal chain: s = o2 - D + H(y-1) + F'(x-1) ---
                    # (s overwrites t in sut; tanh then maps it back to t)
                    for lo, hi in QORD:
                        v.tensor_sub(t_(lo, hi), sl(o2t, lo, hi), sl(tA, lo, hi))
                        v.tensor_add(t_(lo, hi), t_(lo, hi), ev(lo, hi))
                        v.tensor_add(t_(lo, hi), t_(lo, hi), sl(tC2, lo, hi))
                        if not last:
                            act.activation(
                                t_(lo, hi), t_(lo, hi), AF.Tanh, scale=0.5
                            )
                        else:
                            nc.sync.dma_start(
                                out=vu(out_d)[:, lo:hi, :], in_=t_(lo, hi)
                            )

    nc.compile()
    return nc


def kernel(o, vector_field, nabla_w, div_w):
    global LAST_RESULTS
    if "nc" not in _CACHE:
        _CACHE["nc"] = _build()
    nc = _CACHE["nc"]

    o2 = (2.0 * np.asarray(o, dtype=np.float32)[:, 0]).astype(np.float16)
    vf = np.asarray(vector_field, dtype=np.float32)
    s = np.float32(1.0 / np.sqrt(2.0))
    g0f = vf[:, :, 0] * s
    g1f = vf[:, :, 1] * s
    g0 = g0f.astype(np.float16)
    # the kernel reconstructs g1 = gs - g0 on-chip, so build gs to make
    # that reconstruction exact in fp16: gs = fl16(g0) + fl16(g1)
    gs = (g0.astype(np.float32) + g1f.astype(np.float16).astype(np.float32)
          ).astype(np.float16)
    g1df = np.zeros_like(g1f)
    g1df[1:] = g1f[:-1]
    g1d = g1df.astype(np.float16)

    km = np.ascontiguousarray(-(gs[1023:1024, :].astype(np.float32)
                                - g0[1023:1024, :].astype(np.float32))
                              ).astype(np.float16)  # K at the t(1024)=-1 pad

    in_maps = [
        {
            "o2": np.ascontiguousarray(o2[b]),
            "g0": g0,
            "gs": gs,
            "g1d": g1d,
            "km": km,
        }
        for b in range(B)
    ]
    res = bass_utils.run_bass_kernel_spmd(nc, in_maps, core_ids=list(range(B)))
    LAST_RESULTS = res
    return np.stack([r["out"] for r in res.results]).astype(np.float32)


# revision 27
# speedup vs baseline: 2.9701x; 1.2348x over previous
"""Trainium2 Bass kernel for the Chambolle-Pock-style primal/dual stencil loop.

Math (per image, H=W=1024, EPS=0.5, TAU=0.5, 10 iterations):
    u = sigmoid(o/EPS); q = 0
    repeat 10x:
        q  = relu(q - TAU*(vf1*Dy(u) + vf0*Dx(u)))   # forward diffs, zero pad
        Tq = BDy(vf1*q) + BDx(vf0*q)                  # backward diffs, zero pad
        u  = sigmoid((o - Tq)/EPS)
    return (o - Tq)/EPS

Rescaled (qh = 2*sqrt(2)*q, g = vf/sqrt(2), o2 = 2*o, t = tanh(s/2) with
s = 2(o - Tq); u-padding 0 becomes t-padding -1) and with every y-shifted
product rewritten through a host-preshifted field (gs = g0+g1, g1d(y) =
g1(y-1)) so each product depends on exactly one tanh/relu quarter:
    K = g1d*t; H = g1*qh                  # then B(y)=g1(y)*t(y+1) = K(y+1)
    dual:   qh = relu(qh + gs*t - K(y+1) - g0*t(x+1))
    primal: s  = o2 - gs*qh + H(y-1) + (g0*qh)(x-1)
    t = tanh(s/2); output = s of the last iteration.

State and products are fp16 (DVE tensor_tensor runs at 2 elem/cyc for packed
2-byte operands; bf16 fails the 2e-2 gate because the relu makes isolated
pixels chaotic under per-step rounding noise — measured rel-L2 ~1e-2 for
fp16 vs ~3e-2 for bf16).  The primal accumulation happens in fp32 PSUM.

Engine split (four compute engines + DMA queues, all busy):
  - DVE: the A/K/H/D products and the dual-chain accumulations, all
    4B-aligned fp16 tensor_tensor at 2 elem/cycle.
  - GPSIMD: the 2B-misaligned product C~ = g0*t(x+1) (GPSIMD is
    alignment-indifferent) and 3 of 4 quarters of F' = g0*qh.
  - PE (tensor engine): the whole primal chain as accumulating +-identity
    matmuls into PSUM, one 512-column matmul per PSUM bank:
    s = I*o2 + I*H(y-1) - I*(gs*qh) + I*F'(x-1), the x-1 shift expressed
    directly via column-offset PSUM writes (s(0) keeps no F term).
  - Act: relu (SBUF) and tanh (PSUM -> SBUF fp16), one quarter at a time;
    on the last iteration it instead copies s from PSUM for the output DMA.
  - DMA: the K/H boundary rows move between partitions via tiny SBUF->SBUF
    copies whose consumers sit half an iteration away.

Layout: image row y = 8*p + i -> partition p (0..127), plane i (0..7).
Everything operates at quarter (2-plane) granularity in fixed order
(1,2,3,0); per iteration the DVE does ~58 plane-passes at ~0.56us, GPSIMD
14 at 2.05us, PE 64 x 512-col matmuls (~15us), Act 16 quarter-activations.
Working set (~180KB/partition) is SBUF resident: HBM traffic is one 8MB
fp16 load + 2MB store per core.

Sharding: pure data parallel, one image per NeuronCore (B=8 over 8 cores),
g-fields broadcast.
"""

import numpy as np

import concourse.bacc as bacc
import concourse.mybir as mybir
from concourse.tile import TileContext
from concourse import bass_utils

F16 = mybir.dt.float16
F32 = mybir.dt.float32
AF = mybir.ActivationFunctionType

B, H, W = 8, 1024, 1024
P = 128          # SBUF partitions
NP = H // P      # planes per partition = 8
WG = W + 2       # t-plane width incl. guard column (even, keeps 4B align)
MAXITER = 10
QORD = ((2, 4), (4, 6), (6, 8), (0, 2))   # quarter order 1,2,3,0
BK = 512         # PSUM bank = 512 fp32 = one matmul's max output

_CACHE = {}
LAST_RESULTS = None  # BassKernelResults of the most recent run (for test.py)


def _build(reps=1):
    """Build the Bass program.  reps>1 repeats the whole computation (state
    re-initialized each rep, same output) — used only for wall-clock timing
    of the HW kernel when no NTFF profiling is available."""
    nc = bacc.Bacc("TRN2", target_bir_lowering=False, debug=False)

    o2_d = nc.dram_tensor("o2", [H, W], F16, kind="ExternalInput").ap()
    g0_d = nc.dram_tensor("g0", [H, W], F16, kind="ExternalInput").ap()
    gs_d = nc.dram_tensor("gs", [H, W], F16, kind="ExternalInput").ap()
    g1d_d = nc.dram_tensor("g1d", [H, W], F16, kind="ExternalInput").ap()
    km_d = nc.dram_tensor("km", [1, W], F16, kind="ExternalInput").ap()
    id_d = nc.dram_tensor("ident", [P, P], F16, kind="ExternalInput").ap()
    nid_d = nc.dram_tensor("nident", [P, P], F16, kind="ExternalInput").ap()
    out_d = nc.dram_tensor("out", [H, W], F16, kind="ExternalOutput").ap()

    # (H, W) -> (p, i, x) with y = 8*p + i
    def vu(ap):
        return ap.rearrange("(p i) x -> p i x", i=NP)

    v = nc.vector
    gp = nc.gpsimd
    act = nc.scalar
    pe = nc.tensor

    with TileContext(nc) as tc:
        with tc.tile_pool(name="main", bufs=1) as pool:
            o2t = pool.tile([P, NP, W], F16)
            g0t = pool.tile([P, NP, W], F16)
            g1t = pool.tile([P, NP, W], F16)
            gst = pool.tile([P, NP, W], F16)
            g1dt = pool.tile([P, NP, W], F16)
            # t state: planes 0..7, col W = -1 guard for x+1 reads
            sut = pool.tile([P, NP, WG], F16)
            qht = pool.tile([P, NP, W], F16)
            tA = pool.tile([P, NP, W], F16)
            # K = g1d*t at planes 0..7; plane 8 = K[8p+8] boundary
            # (partition 127: constant -g1[1023] = K at the t=-1 pad row)
            tK = pool.tile([P, NP + 1, W], F16)
            # H = g1*qh at planes 1..8; plane 0 = H[8p-1] boundary
            # (partition 0: zero pad)
            tH = pool.tile([P, NP + 1, W], F16)
            tC = pool.tile([P, NP, W], F16)
            idt = pool.tile([P, P], F16)
            nidt = pool.tile([P, P], F16)

            def t_(lo, hi):     # t rows 8p+lo..8p+hi-1
                return sut[:, lo:hi, 0:W]

            def trt(lo, hi):    # t(x+1) (col W = -1 guard)
                return sut[:, lo:hi, 1 : W + 1]

            def bv(lo, hi):     # B(y) = K(y+1) (plane 8 = boundary)
                return tK[:, lo + 1 : hi + 1, :]

            def qh_(lo, hi):
                return qht[:, lo:hi, :]

            def sl(tile, lo, hi):
                return tile[:, lo:hi, :]

            # --- setup ---
            # Only guard regions need init: everything else is written
            # before its first read.  Loads are HBM-bandwidth serial
            # (~1.6us per field quarter), so interleave in need order;
            # g1 = gs - g0 is derived on-chip to cut the load volume.
            v.memset(sut[:, :, W:WG], -1.0)        # x+1 guard column = -1
            v.memset(tH[0:1, 0, :], 0.0)           # H[-1] pad row = 0
            nc.sync.dma_start(out=tK[P - 1 : P, NP, :], in_=km_d)
            nc.sync.dma_start(out=idt[:, :], in_=id_d)
            nc.sync.dma_start(out=nidt[:, :], in_=nid_d)
            for lo, hi in QORD:
                nc.sync.dma_start(
                    out=o2t[:, lo:hi, :], in_=vu(o2_d)[:, lo:hi, :]
                )
                nc.sync.dma_start(
                    out=g0t[:, lo:hi, :], in_=vu(g0_d)[:, lo:hi, :]
                )
                nc.sync.dma_start(
                    out=gst[:, lo:hi, :], in_=vu(gs_d)[:, lo:hi, :]
                )
                nc.sync.dma_start(
                    out=g1dt[:, lo:hi, :], in_=vu(g1d_d)[:, lo:hi, :]
                )

            def dma_kshift():
                # tK[p, 8] = K[8p+8] = tK[p+1, 0]; partition 127 keeps km
                nc.sync.dma_start(
                    out=tK[0 : P - 1, NP, :], in_=tK[1:P, 0, :]
                )

            def dma_hshift():
                # tH[p, 0] = H[8p-1] = tH[p-1, 8]; partition 0 stays 0
                nc.sync.dma_start(
                    out=tH[1:P, 0, :], in_=tH[0 : P - 1, NP, :]
                )

            with tc.tile_pool(name="ps", bufs=2, space="PSUM") as pp:
                for _rep in range(reps):
                    for lo, hi in QORD:
                        act.activation(
                            t_(lo, hi), sl(o2t, lo, hi), AF.Tanh, scale=0.5
                        )

                    for it in range(MAXITER):
                        first = it == 0
                        last = it == MAXITER - 1
                        # --- dual products (each needs one tanh quarter) ---
                        # GPSIMD: C~ = g0*t(x+1) (misaligned read is free here)
                        for lo, hi in QORD:
                            gp.tensor_mul(
                                sl(tC, lo, hi), sl(g0t, lo, hi), trt(lo, hi)
                            )
                        # DVE: A = gs*t, K = g1d*t; g1 = gs - g0 (iter 0 only)
                        for lo, hi in QORD:
                            v.tensor_mul(sl(tA, lo, hi), sl(gst, lo, hi), t_(lo, hi))
                            v.tensor_mul(sl(tK, lo, hi), sl(g1dt, lo, hi), t_(lo, hi))
                            if first:
                                v.tensor_sub(
                                    sl(g1t, lo, hi), sl(gst, lo, hi), sl(g0t, lo, hi)
                                )
                        dma_kshift()
                        # --- dual chain: qh = relu(qh + A - K(y+1) - C~) ---
                        for lo, hi in QORD:
                            if first:
                                v.tensor_sub(qh_(lo, hi), sl(tA, lo, hi), bv(lo, hi))
                            else:
                                v.tensor_add(qh_(lo, hi), qh_(lo, hi), sl(tA, lo, hi))
                                v.tensor_sub(qh_(lo, hi), qh_(lo, hi), bv(lo, hi))
                            v.tensor_sub(qh_(lo, hi), qh_(lo, hi), sl(tC, lo, hi))
                            act.activation(qh_(lo, hi), qh_(lo, hi), AF.Relu)
                        # --- primal products (each needs one relu quarter);
                        # they must ALL precede the PE loop in program order
                        # because the E-view of quarter q reads H rows from
                        # quarter q-1 (including the wrap onto q0, last) ---
                        for lo, hi in QORD:
                            # F' = g0*qh: GPSIMD except quarter q0 (keeps
                            # GPSIMD under its throughput budget)
                            if lo == 0:
                                v.tensor_mul(
                                    sl(tC, lo, hi), sl(g0t, lo, hi), qh_(lo, hi)
                                )
                            else:
                                gp.tensor_mul(
                                    sl(tC, lo, hi), sl(g0t, lo, hi), qh_(lo, hi)
                                )
                            # H = g1*qh (planes 1..8 of tH), D = gs*qh
                            v.tensor_mul(
                                tH[:, lo + 1 : hi + 1, :], sl(g1t, lo, hi),
                                qh_(lo, hi),
                            )
                            if (lo, hi) == (6, 8):
                                dma_hshift()
                            v.tensor_mul(sl(tA, lo, hi), sl(gst, lo, hi), qh_(lo, hi))
                        # --- primal accumulation on PE: s = o2 - D + F'(x-1)
                        # + H(y-1), one matmul per PSUM bank, E-view last so
                        # only the group tail waits on the wrap H quarter;
                        # accumulation groups are tracked per 2KB bank, so
                        # each bank's final matmul carries stop=True ---
                        for lo, hi in QORD:
                            ps = pp.tile([P, 2, W], F32)
                            for p in range(2):
                                for c in (0, BK):
                                    pe.matmul(
                                        ps[:, p, c : c + BK], idt[:, :],
                                        o2t[:, lo + p, c : c + BK],
                                        start=True, stop=False,
                                    )
                            for p in range(2):
                                for c in (0, BK):
                                    pe.matmul(
                                        ps[:, p, c : c + BK], nidt[:, :],
                                        tA[:, lo + p, c : c + BK],
                                        start=False, stop=False,
                                    )
                            for p in range(2):
                                pe.matmul(
                                    ps[:, p, 1:BK], idt[:, :],
                                    tC[:, lo + p, 0 : BK - 1],
                                    start=False, stop=False,
                                )
                                pe.matmul(
                                    ps[:, p, BK:W], idt[:, :],
                                    tC[:, lo + p, BK - 1 : W - 1],
                                    start=False, stop=False,
                                )
                            for p in range(2):
                                for c in (0, BK):
                                    pe.matmul(
                                        ps[:, p, c : c + BK], idt[:, :],
                                        tH[:, lo + p, c : c + BK],
                                        start=False, stop=True,
                                    )
                            if not last:
                                act.activation(
                                    t_(lo, hi), ps[:, :, :], AF.Tanh, scale=0.5
                                )
                            else:
                                act.activation(t_(lo, hi), ps[:, :, :], AF.Copy)
                                nc.sync.dma_start(
                                    out=vu(out_d)[:, lo:hi, :], in_=t_(lo, hi)
                                )

    nc.compile()
    return nc


def kernel(o, vector_field, nabla_w, div_w):
    global LAST_RESULTS
    if "nc" not in _CACHE:
        _CACHE["nc"] = _build()
    nc = _CACHE["nc"]

    o2 = (2.0 * np.asarray(o, dtype=np.float32)[:, 0]).astype(np.float16)
    vf = np.asarray(vector_field, dtype=np.float32)
    s = np.float32(1.0 / np.sqrt(2.0))
    g0f = vf[:, :, 0] * s
    g1f = vf[:, :, 1] * s
    g0 = g0f.astype(np.float16)
    # the kernel reconstructs g1 = gs - g0 on-chip, so build gs to make
    # that reconstruction exact-ish in fp16: gs = fl16(g0) + fl16(g1)
    gs = (g0.astype(np.float32) + g1f.astype(np.float16).astype(np.float32)
          ).astype(np.float16)
    g1df = np.zeros_like(g1f)
    g1df[1:] = g1f[:-1]
    g1d = g1df.astype(np.float16)
    km = np.ascontiguousarray(-(gs[1023:1024, :].astype(np.float32)
                                - g0[1023:1024, :].astype(np.float32))
                              ).astype(np.float16)  # K at the t(1024)=-1 pad
    ident = np.eye(P, dtype=np.float16)

    in_maps = [
        {
            "o2": np.ascontiguousarray(o2[b]),
            "g0": g0,
            "gs": gs,
            "g1d": g1d,
            "km": km,
            "ident": ident,
            "nident": -ident,
        }
        for b in range(B)
    ]
    res = bass_utils.run_bass_kernel_spmd(nc, in_maps, core_ids=list(range(B)))
    LAST_RESULTS = res
    return np.stack([r["out"] for r in res.results]).astype(np.float32)


# revision 32
# speedup vs baseline: 3.6143x; 1.2169x over previous
"""Trainium2 Bass kernel for the Chambolle-Pock-style primal/dual stencil loop.

Math (per image, H=W=1024, EPS=0.5, TAU=0.5, 10 iterations):
    u = sigmoid(o/EPS); q = 0
    repeat 10x:
        q  = relu(q - TAU*(vf1*Dy(u) + vf0*Dx(u)))   # forward diffs, zero pad
        Tq = BDy(vf1*q) + BDx(vf0*q)                  # backward diffs, zero pad
        u  = sigmoid((o - Tq)/EPS)
    return (o - Tq)/EPS

Rescaled (qh = 2*sqrt(2)*q, g = vf/sqrt(2), o2 = 2*o, t = tanh(s/2) with
s = 2(o - Tq); u-padding 0 becomes t-padding -1) and with every y-shifted
product rewritten through a host-preshifted field (gs = g0+g1, g1d(y) =
g1(y-1)) so each product depends on exactly one tanh/relu quarter:
    K = g1d*t; H = g1*qh                  # then B(y)=g1(y)*t(y+1) = K(y+1)
    dual:   qh = relu(qh + gs*t - K(y+1) - g0*t(x+1))
    primal: s  = o2 - gs*qh + H(y-1) + (g0*qh)(x-1)
    t = tanh(s/2); output = s of the last iteration.

State and products are fp16 (DVE tensor_tensor runs at 2 elem/cyc for packed
2-byte operands; bf16 fails the 2e-2 gate because the relu makes isolated
pixels chaotic under per-step rounding noise — measured rel-L2 ~1e-2 for
fp16 vs ~3e-2 for bf16).  The primal accumulation happens in fp32 PSUM.

Engine split (four compute engines + DMA queues, all busy):
  - DVE: the A/K/H/D products and the dual-chain accumulations, all
    4B-aligned fp16 tensor_tensor at 2 elem/cycle.
  - GPSIMD: the 2B-misaligned product C~ = g0*t(x+1) (GPSIMD is
    alignment-indifferent) and 3 of 4 quarters of F' = g0*qh.
  - PE (tensor engine): the whole primal chain as accumulating +-identity
    matmuls into PSUM, one 512-column matmul per PSUM bank:
    s = I*o2 + I*H(y-1) - I*(gs*qh) + I*F'(x-1), the x-1 shift expressed
    directly via column-offset PSUM writes (s(0) keeps no F term).
  - Act: relu (SBUF) and tanh (PSUM -> SBUF fp16), one quarter at a time;
    on the last iteration it instead copies s from PSUM for the output DMA.
  - DMA: the K/H boundary rows move between partitions via tiny SBUF->SBUF
    copies whose consumers sit half an iteration away.

Layout: image row y = 8*p + i -> partition p (0..127), plane i (0..7).
Everything operates at quarter (2-plane) granularity in fixed order
(1,2,3,0); per iteration the DVE does ~58 plane-passes at ~0.56us, GPSIMD
14 at 2.05us, PE 64 x 512-col matmuls (~15us), Act 16 quarter-activations.
Working set (~180KB/partition) is SBUF resident: HBM traffic is one 8MB
fp16 load + 2MB store per core.

Sharding: pure data parallel, one image per NeuronCore (B=8 over 8 cores),
g-fields broadcast.
"""

import numpy as np

import concourse.bacc as bacc
import concourse.mybir as mybir
from concourse.tile import TileContext
from concourse import bass_utils

F16 = mybir.dt.float16
F32 = mybir.dt.float32
AF = mybir.ActivationFunctionType

B, H, W = 8, 1024, 1024
P = 128          # SBUF partitions
NP = H // P      # planes per partition = 8
WG = W + 2       # t-plane width incl. guard column (even, keeps 4B align)
MAXITER = 10
QORD = ((2, 4), (4, 6), (6, 8), (0, 2))   # quarter order 1,2,3,0
BK = 512         # PSUM bank = 512 fp32 = one matmul's max output

_CACHE = {}
LAST_RESULTS = None  # BassKernelResults of the most recent run (for test.py)


def _build(reps=1):
    """Build the Bass program.  reps>1 repeats the whole computation (state
    re-initialized each rep, same output) — used only for wall-clock timing
    of the HW kernel when no NTFF profiling is available."""
    nc = bacc.Bacc("TRN2", target_bir_lowering=False, debug=False)

    o2_d = nc.dram_tensor("o2", [H, W], F16, kind="ExternalInput").ap()
    g0_d = nc.dram_tensor("g0", [H, W], F16, kind="ExternalInput").ap()
    gs_d = nc.dram_tensor("gs", [H, W], F16, kind="ExternalInput").ap()
    g1d_d = nc.dram_tensor("g1d", [H, W], F16, kind="ExternalInput").ap()
    km_d = nc.dram_tensor("km", [1, W], F16, kind="ExternalInput").ap()
    id_d = nc.dram_tensor("ident", [P, P], F16, kind="ExternalInput").ap()
    nid_d = nc.dram_tensor("nident", [P, P], F16, kind="ExternalInput").ap()
    out_d = nc.dram_tensor("out", [H, W], F16, kind="ExternalOutput").ap()

    # (H, W) -> (p, i, x) with y = 8*p + i
    def vu(ap):
        return ap.rearrange("(p i) x -> p i x", i=NP)

    v = nc.vector
    gp = nc.gpsimd
    act = nc.scalar
    pe = nc.tensor

    with TileContext(nc) as tc:
        with tc.tile_pool(name="main", bufs=1) as pool:
            o2t = pool.tile([P, NP, W], F16)
            g0t = pool.tile([P, NP, W], F16)
            g1t = pool.tile([P, NP, W], F16)
            gst = pool.tile([P, NP, W], F16)
            g1dt = pool.tile([P, NP, W], F16)
            # t state: planes 0..7, col W = -1 guard for x+1 reads
            sut = pool.tile([P, NP, WG], F16)
            qht = pool.tile([P, NP, W], F16)
            tA = pool.tile([P, NP, W], F16)
            # K = g1d*t at planes 0..7; plane 8 = K[8p+8] boundary
            # (partition 127: constant -g1[1023] = K at the t=-1 pad row)
            tK = pool.tile([P, NP + 1, W], F16)
            # H = g1*qh at planes 1..8; plane 0 = H[8p-1] boundary
            # (partition 0: zero pad)
            tH = pool.tile([P, NP + 1, W], F16)
            tC = pool.tile([P, NP, W], F16)
            idt = pool.tile([P, P], F16)
            nidt = pool.tile([P, P], F16)

            def t_(lo, hi):     # t rows 8p+lo..8p+hi-1
                return sut[:, lo:hi, 0:W]

            def trt(lo, hi):    # t(x+1) (col W = -1 guard)
                return sut[:, lo:hi, 1 : W + 1]

            def bv(lo, hi):     # B(y) = K(y+1) (plane 8 = boundary)
                return tK[:, lo + 1 : hi + 1, :]

            def qh_(lo, hi):
                return qht[:, lo:hi, :]

            def sl(tile, lo, hi):
                return tile[:, lo:hi, :]

            # --- setup ---
            # Only guard regions need init: everything else is written
            # before its first read.  Loads are HBM-bandwidth serial
            # (~1.6us per field quarter), so interleave in need order;
            # g1 = gs - g0 is derived on-chip to cut the load volume.
            v.memset(sut[:, :, W:WG], -1.0)        # x+1 guard column = -1
            v.memset(tH[0:1, 0, :], 0.0)           # H[-1] pad row = 0
            nc.sync.dma_start(out=tK[P - 1 : P, NP, :], in_=km_d)
            nc.sync.dma_start(out=idt[:, :], in_=id_d)
            nc.sync.dma_start(out=nidt[:, :], in_=nid_d)
            for lo, hi in QORD:
                nc.sync.dma_start(
                    out=o2t[:, lo:hi, :], in_=vu(o2_d)[:, lo:hi, :]
                )
                nc.sync.dma_start(
                    out=g0t[:, lo:hi, :], in_=vu(g0_d)[:, lo:hi, :]
                )
                nc.sync.dma_start(
                    out=gst[:, lo:hi, :], in_=vu(gs_d)[:, lo:hi, :]
                )
                nc.sync.dma_start(
                    out=g1dt[:, lo:hi, :], in_=vu(g1d_d)[:, lo:hi, :]
                )

            def dma_kshift():
                # tK[p, 8] = K[8p+8] = tK[p+1, 0]; partition 127 keeps km
                nc.sync.dma_start(
                    out=tK[0 : P - 1, NP, :], in_=tK[1:P, 0, :]
                )

            def dma_hshift():
                # tH[p, 0] = H[8p-1] = tH[p-1, 8]; partition 0 stays 0
                nc.sync.dma_start(
                    out=tH[1:P, 0, :], in_=tH[0 : P - 1, NP, :]
                )

            with tc.tile_pool(name="ps", bufs=2, space="PSUM") as pp:

                def alloc_ps():
                    # single allocation site -> one rotating pair of 4-bank
                    # PSUM buffers shared by the dual and primal chains
                    return pp.tile([P, 2, W], F32, name="ps")

                for _rep in range(reps):
                    for lo, hi in QORD:
                        act.activation(
                            t_(lo, hi), sl(o2t, lo, hi), AF.Tanh, scale=0.5
                        )

                    for it in range(MAXITER):
                        first = it == 0
                        last = it == MAXITER - 1
                        # --- dual products (each needs one tanh quarter) ---
                        # GPSIMD: C~ = g0*t(x+1) (misaligned read is free here)
                        for lo, hi in QORD:
                            gp.tensor_mul(
                                sl(tC, lo, hi), sl(g0t, lo, hi), trt(lo, hi)
                            )
                        # DVE: A = gs*t, K = g1d*t; g1 = gs - g0 (iter 0 only)
                        for lo, hi in QORD:
                            v.tensor_mul(sl(tA, lo, hi), sl(gst, lo, hi), t_(lo, hi))
                            v.tensor_mul(sl(tK, lo, hi), sl(g1dt, lo, hi), t_(lo, hi))
                            if first:
                                v.tensor_sub(
                                    sl(g1t, lo, hi), sl(gst, lo, hi), sl(g0t, lo, hi)
                                )
                        dma_kshift()
                        # --- dual chain on PE: qh = relu(qh + A - K(y+1)
                        # - C~) as +-identity matmuls into fp32 PSUM, then
                        # relu reads PSUM back to SBUF fp16 on Act ---
                        for lo, hi in QORD:
                            ps = alloc_ps()
                            srcs = [] if first else [(idt, qht, lo)]
                            srcs += [(idt, tA, lo), (nidt, tK, lo + 1), (nidt, tC, lo)]
                            for i_s, (w, tile, plo) in enumerate(srcs):
                                for p in range(2):
                                    for c in (0, BK):
                                        pe.matmul(
                                            ps[:, p, c : c + BK], w[:, :],
                                            tile[:, plo + p, c : c + BK],
                                            start=(i_s == 0),
                                            stop=(i_s == len(srcs) - 1),
                                        )
                            act.activation(qh_(lo, hi), ps[:, :, :], AF.Relu)
                        # --- primal products (each needs one relu quarter);
                        # they must ALL precede the PE loop in program order
                        # because the E-view of quarter q reads H rows from
                        # quarter q-1 (including the wrap onto q0, last) ---
                        for lo, hi in QORD:
                            # F' = g0*qh: quarter q1 on GPSIMD, rest on DVE
                            # (keeps both engines at ~21us/iter of products)
                            if lo == 2:
                                gp.tensor_mul(
                                    sl(tC, lo, hi), sl(g0t, lo, hi), qh_(lo, hi)
                                )
                            else:
                                v.tensor_mul(
                                    sl(tC, lo, hi), sl(g0t, lo, hi), qh_(lo, hi)
                                )
                            # H = g1*qh (planes 1..8 of tH), D = gs*qh
                            v.tensor_mul(
                                tH[:, lo + 1 : hi + 1, :], sl(g1t, lo, hi),
                                qh_(lo, hi),
                            )
                            if (lo, hi) == (6, 8):
                                dma_hshift()
                            v.tensor_mul(sl(tA, lo, hi), sl(gst, lo, hi), qh_(lo, hi))
                        # --- primal accumulation on PE: s = o2 - D + F'(x-1)
                        # + H(y-1), one matmul per PSUM bank, E-view last so
                        # only the group tail waits on the wrap H quarter;
                        # accumulation groups are tracked per 2KB bank, so
                        # each bank's final matmul carries stop=True ---
                        for lo, hi in QORD:
                            ps = alloc_ps()
                            for p in range(2):
                                for c in (0, BK):
                                    pe.matmul(
                                        ps[:, p, c : c + BK], idt[:, :],
                                        o2t[:, lo + p, c : c + BK],
                                        start=True, stop=False,
                                    )
                            for p in range(2):
                                for c in (0, BK):
                                    pe.matmul(
                                        ps[:, p, c : c + BK], nidt[:, :],
                                        tA[:, lo + p, c : c + BK],
                                        start=False, stop=False,
                                    )
                            for p in range(2):
                                pe.matmul(
                                    ps[:, p, 1:BK], idt[:, :],
                                    tC[:, lo + p, 0 : BK - 1],
                                    start=False, stop=False,
                                )
                                pe.matmul(
                                    ps[:, p, BK:W], idt[:, :],
                                    tC[:, lo + p, BK - 1 : W - 1],
                                    start=False, stop=False,
                                )
                            for p in range(2):
                                for c in (0, BK):
                                    pe.matmul(
                                        ps[:, p, c : c + BK], idt[:, :],
                                        tH[:, lo + p, c : c + BK],
                                        start=False, stop=True,
                                    )
                            if not last:
                                act.activation(
                                    t_(lo, hi), ps[:, :, :], AF.Tanh, scale=0.5
                                )
                            else:
                                act.activation(t_(lo, hi), ps[:, :, :], AF.Copy)
                                nc.sync.dma_start(
                                    out=vu(out_d)[:, lo:hi, :], in_=t_(lo, hi)
                                )

    nc.compile()
    return nc


def kernel(o, vector_field, nabla_w, div_w):
    global LAST_RESULTS
    if "nc" not in _CACHE:
        _CACHE["nc"] = _build()
    nc = _CACHE["nc"]

    o2 = (2.0 * np.asarray(o, dtype=np.float32)[:, 0]).astype(np.float16)
    vf = np.asarray(vector_field, dtype=np.float32)
    s = np.float32(1.0 / np.sqrt(2.0))
    g0f = vf[:, :, 0] * s
    g1f = vf[:, :, 1] * s
    g0 = g0f.astype(np.float16)
    # the kernel reconstructs g1 = gs - g0 on-chip, so build gs to make
    # that reconstruction exact-ish in fp16: gs = fl16(g0) + fl16(g1)
    gs = (g0.astype(np.float32) + g1f.astype(np.float16).astype(np.float32)
          ).astype(np.float16)
    g1df = np.zeros_like(g1f)
    g1df[1:] = g1f[:-1]
    g1d = g1df.astype(np.float16)
    km = np.ascontiguousarray(-(gs[1023:1024, :].astype(np.float32)
                                - g0[1023:1024, :].astype(np.float32))
                              ).astype(np.float16)  # K at the t(1024)=-1 pad
    ident = np.eye(P, dtype=np.float16)

    in_maps = [
        {
            "o2": np.ascontiguousarray(o2[b]),
            "g0": g0,
            "gs": gs,
            "g1d": g1d,
            "km": km,
            "ident": ident,
            "nident": -ident,
        }
        for b in range(B)
    ]
    res = bass_utils.run_bass_kernel_spmd(nc, in_maps, core_ids=list(range(B)))
    LAST_RESULTS = res
    return np.stack([r["out"] for r in res.results]).astype(np.float32)
